# revision 17
# baseline (speedup 1.0000x reference)
"""Multi-head attention TRN2 Bass kernel, v3 (bf16, fully SBUF-resident).

Problem: B=4, S=2048, D=1024, H=16 heads (DK=64), fp32 reference, random
0/1 attention mask broadcast over heads.

Sharding: 8 cores = (batch b, query-half) pairs.  Core c handles batch
c//2, query rows [(c%2)*1024, (c%2+1)*1024).  K/V projections for the
batch are computed redundantly on the 2 cores sharing a batch; no
collectives, each core writes a disjoint output slice.

v3 design (vs the v1 fp32r DRAM-spilled baseline):
  - All matmul operands bf16 (1 cyc/row like fp32r, half the SBUF/DMA).
    Everything lives in SBUF; no DRAM spills of qh/kh/ct.
  - Head-pair (j) software pipelining: projections for pair j+1 are
    emitted as filler psum-groups interleaved into pair j's attention
    chunk loop, so PE streams projections while ACT/DVE chew on
    exp/mask.  W_v slices span two pairs (256-wide streams) to amortize
    LDWEIGHTS of the vT-stationary v-projection.
  - Algebraic bias folds: k-bias shifts every score of a q-row equally
    -> softmax-invariant -> dropped.  v-bias passes through attention
    (weights sum to 1) -> folded into b_o on host (b_o' = b_v @ W_o +
    b_o).  1/sqrt(dk) and q-bias folded into W_q/b_q on host.
  - Attention (per head): scoresT[k,q] chunks [128,1024] on 2 psum
    banks; exp on ACT with no max-subtraction (scaled scores are O(1));
    mask applied multiplicatively post-exp on DVE in bf16 (2x mode);
    AV keeps d-on-partitions: pa[65, 1024] += [vh|1].T @ mexpT, vh
    stationary (65-column LDWEIGHTS, 512-row streams), denominator in
    row 64.  HW rule respected throughout: at most one open psum
    accumulation group per 2KB bank.
  - Normalize: DVE reciprocal of the denominator row, GPSIMD
    partition_broadcast (DVE cannot broadcast along partitions), DVE
    multiply straight into ctT -- no DMA round trip.
  - Output projection: stationary ctT slice shared by consecutive
    matmuls (d outer), bias via ones-row matmul, ACT drain, DMA out.

PSUM budget (8 banks): scores 2x[128,1024]f32 (4) + AV accum
1x[65,1024]f32 (2) + proj 2x[128,512] (2).
"""

import os
import sys

if "/opt/trn_rl_repo" not in sys.path:
    sys.path.insert(0, "/opt/trn_rl_repo")
os.environ.setdefault("MYCRO_LOCAL_CACHE", "1")

import numpy as np
import ml_dtypes

import concourse.bass as bass
import concourse.bacc as bacc
import concourse.mybir as mybir
import concourse.tile as tile
from concourse.bass import ts

B, S, D, H, DK = 4, 2048, 1024, 16, 64
SQ = S // 2          # q rows per core
P = 128
NC_K = S // P        # 16 k-chunks
NJ = D // P          # 8 feature chunks = head pairs
NQT = SQ // P        # 8 q tiles
N_CORES = 8

F32 = mybir.dt.float32
BF16 = mybir.dt.bfloat16
AF = mybir.ActivationFunctionType


def build_program():
    nc = bacc.Bacc(
        "TRN2",
        target_bir_lowering=False,
        debug=False,
        enable_asserts=False,
    )

    # ---- DRAM I/O (per-core slices; host pre-transposed, bf16) ----
    qT_d = nc.dram_tensor("qT", [D, SQ], BF16, kind="ExternalInput").ap()
    kT_d = nc.dram_tensor("kT", [D, S], BF16, kind="ExternalInput").ap()
    vT_d = nc.dram_tensor("vT", [D, S], BF16, kind="ExternalInput").ap()
    mT_d = nc.dram_tensor("maskT", [S, SQ], BF16, kind="ExternalInput").ap()
    wq_d = nc.dram_tensor("wq", [D, D], BF16, kind="ExternalInput").ap()
    wk_d = nc.dram_tensor("wk", [D, D], BF16, kind="ExternalInput").ap()
    wv_d = nc.dram_tensor("wv", [D, D], BF16, kind="ExternalInput").ap()
    wo_d = nc.dram_tensor("wo", [D, D], BF16, kind="ExternalInput").ap()
    bq_d = nc.dram_tensor("bq", [D], F32, kind="ExternalInput").ap()
    bo_d = nc.dram_tensor("bo", [D], BF16, kind="ExternalInput").ap()
    ones_d = nc.dram_tensor("ones_row", [1, P], BF16, kind="ExternalInput").ap()
    out_d = nc.dram_tensor("out", [SQ, D], F32, kind="ExternalOutput").ap()

    with tile.TileContext(nc) as tc:
        _build(nc, tc, qT_d, kT_d, vT_d, mT_d, wq_d, wk_d, wv_d, wo_d,
               bq_d, bo_d, ones_d, out_d)

    nc.compile()
    return nc


def _build(nc, tc, qT_d, kT_d, vT_d, mT_d, wq_d, wk_d, wv_d, wo_d,
           bq_d, bo_d, ones_d, out_d):
    from contextlib import ExitStack

    with ExitStack() as top:
        # ---------------- persistent SBUF ----------------
        consts = top.enter_context(tc.tile_pool(name="consts", bufs=1))
        ones1 = consts.tile([1, P], BF16, tag="ones1")
        bo_row = consts.tile([1, D], BF16, tag="bo_row")
        bq_sb = consts.tile([P, NJ], F32, tag="bq_sb")

        inp = top.enter_context(tc.tile_pool(name="inp", bufs=1))
        qT = [inp.tile([P, SQ], BF16, tag=f"qT{d}", name=f"qT{d}")
              for d in range(NJ)]
        kT = [inp.tile([P, S], BF16, tag=f"kT{d}", name=f"kT{d}")
              for d in range(NJ)]
        vT = [inp.tile([P, S], BF16, tag=f"vT{d}", name=f"vT{d}")
              for d in range(NJ)]
        maskT = [inp.tile([P, SQ], BF16, tag=f"mT{c}", name=f"mT{c}")
                 for c in range(NC_K)]
        wo_sb = [inp.tile([P, D], BF16, tag=f"wo{d}", name=f"wo{d}")
                 for d in range(NJ)]
        ctT = [inp.tile([P, SQ], BF16, tag=f"ctT{d}", name=f"ctT{d}")
               for d in range(NJ)]

        # ---------------- ring pools (head-pair pipelined) ----------------
        wsl = top.enter_context(tc.tile_pool(name="wsl", bufs=2))
        qh_p = top.enter_context(tc.tile_pool(name="qh", bufs=2))
        kh_p = top.enter_context(tc.tile_pool(name="kh", bufs=2))
        vh_p = top.enter_context(tc.tile_pool(name="vh", bufs=2))
        et_p = top.enter_context(tc.tile_pool(name="et", bufs=2))
        mt_p = top.enter_context(tc.tile_pool(name="mt", bufs=3))
        rc_p = top.enter_context(tc.tile_pool(name="rc", bufs=1))
        bc_p = top.enter_context(tc.tile_pool(name="bc", bufs=1))
        so_p = top.enter_context(tc.tile_pool(name="so", bufs=1))

        ps_scr = top.enter_context(
            tc.tile_pool(name="ps_scr", bufs=2, space="PSUM"))
        ps_av = top.enter_context(
            tc.tile_pool(name="ps_av", bufs=1, space="PSUM"))
        ps_pr = top.enter_context(
            tc.tile_pool(name="ps_pr", bufs=2, space="PSUM"))

        # W slice view: DRAM W[1024, 1024] rows (dc p), cols j*128+f
        # -> SBUF [p, dc, f] = [128, 8, width]
        def w_slice_ap(w_d, j, width=P):
            return w_d.rearrange("(dc p) f -> p dc f", p=P)[
                :, :, j * P: j * P + width]

        state = {}

        def emit_wdma(j):
            """Prefetch W_q/W_k slices for pair j; W_v slice covers pairs
            (j, j+1) and is fetched on even j only (v-proj streams 256-wide
            to amortize LDWEIGHTS of the vT-stationary)."""
            wq_t = wsl.tile([P, D], BF16, tag="wq_sl")
            wk_t = wsl.tile([P, D], BF16, tag="wk_sl")
            nc.sync.dma_start(
                wq_t[:].rearrange("p (dc f) -> p dc f", f=P),
                w_slice_ap(wq_d, j))
            nc.sync.dma_start(
                wk_t[:].rearrange("p (dc f) -> p dc f", f=P),
                w_slice_ap(wk_d, j))
            if j % 2 == 0:
                wv_t = wsl.tile([P, 2 * D], BF16, tag="wv_sl")
                nc.sync.dma_start(
                    wv_t[:].rearrange("p (dc f) -> p dc f", f=2 * P),
                    w_slice_ap(wv_d, j, width=2 * P))
            else:
                wv_t = None
            state[("w", j)] = (wq_t, wk_t, wv_t)

        def proj_groups(j):
            """Projections for pair j as a list of psum-group closures, so
            they can be interleaved into the previous pair's attention.
            Even j also produces pair j+1's v-projection (256-wide)."""
            wq_t, wk_t, wv_t = state.pop(("w", j))
            qh = qh_p.tile([P, SQ], BF16, tag="qh")
            kh = kh_p.tile([P, S], BF16, tag="kh")
            if j % 2 == 0:
                vh = vh_p.tile([P, NC_K * 2 * (DK + 1)], BF16, tag="vha",
                               name=f"vha_{j}")
                vh1 = vh_p.tile([P, NC_K * 2 * (DK + 1)], BF16, tag="vha",
                                name=f"vha_{j + 1}")
                # ones columns (denominator trick), one memset per pair
                for t in (vh, vh1):
                    nc.gpsimd.memset(
                        t[:].rearrange("p (n w) -> p n w", w=DK + 1)
                        [:, :, DK:DK + 1], 1.0)
                state[("vh", j + 1)] = vh1
            else:
                vh = state.pop(("vh", j))
                vh1 = None
            state[j] = (qh, kh, vh)
            groups = []

            # q-proj: d outer over both 512-halves so consecutive matmuls
            # share the stationary W slice (one LDWEIGHTS per d on HW);
            # the two open accumulation groups sit in separate banks.
            def qgroup():
                pqs = [ps_pr.tile([P, 512], F32, tag="ppr", name=f"pq{i}")
                       for i in range(2)]
                for d in range(NJ):
                    for half in range(2):
                        nc.tensor.matmul(
                            pqs[half][:], wq_t[:, ts(d, P)],
                            qT[d][:, ts(half, 512)],
                            start=(d == 0), stop=(d == NJ - 1))
                for half in range(2):
                    nc.vector.tensor_scalar_add(
                        qh[:, ts(half, 512)], pqs[half][:], bq_sb[:, ts(j, 1)])

            # k-proj: same d-outer pattern per pair of 512-halves
            # (no bias: softmax-invariant)
            def kgroup(hp):
                pks = [ps_pr.tile([P, 512], F32, tag="ppr", name=f"pk{i}")
                       for i in range(2)]
                for d in range(NJ):
                    for hx in range(2):
                        nc.tensor.matmul(
                            pks[hx][:], wk_t[:, ts(d, P)],
                            kT[d][:, ts(2 * hp + hx, 512)],
                            start=(d == 0), stop=(d == NJ - 1))
                for hx in range(2):
                    nc.vector.tensor_copy(
                        kh[:, ts(2 * hp + hx, 512)], pks[hx][:])

            # v-proj (even j): out[kpos 128, 256 feat spanning pairs
            # j, j+1] per k-chunk; 2 sequential psum groups per tile (one
            # open accumulation group per bank).  v-bias folded into b_o.
            def vgroup(g):
                pv = ps_pr.tile([P, 512], F32, tag="ppr")
                for cc in range(2):
                    c = g * 2 + cc
                    for d in range(NJ):
                        nc.tensor.matmul(
                            pv[:, cc * 256:(cc + 1) * 256],
                            vT[d][:, ts(c, P)],
                            wv_t[:, d * 256:(d + 1) * 256],
                            start=(d == 0), stop=(d == NJ - 1))
                    for pj, vdst in ((0, vh), (1, vh1)):
                        dst = vdst[:].rearrange("p (n w) -> p n w", w=DK + 1)
                        seg = pv[:, cc * 256 + pj * P: cc * 256 + (pj + 1) * P]
                        nc.vector.tensor_copy(
                            dst[:, 2 * c: 2 * c + 2, 0:DK],
                            seg.rearrange("p (h w) -> p h w", w=DK))

            import functools
            for hp in range(S // 1024):
                groups.append(functools.partial(kgroup, hp))
            groups.append(qgroup)
            if j % 2 == 0:
                for g in range(NC_K // 2):
                    groups.append(functools.partial(vgroup, g))
            return groups

        def emit_attn(j, fillers=()):
            """Attention for pair j; fillers = next pair's proj groups,
            interleaved into the chunk stream to keep PE fed while ACT/DVE
            work on exp/mask."""
            fillers = list(fillers)
            qh, kh, vh = state.pop(j)
            nfill = len(fillers)
            fi = 0
            for hh in range(2):
                # pa rows 0..63 = unnormalized out^T, row 64 = softmax
                # denominators (vh column 64 is all-ones); the two 512-wide
                # halves are separate accumulation groups, one per bank.
                pa = ps_av.tile([DK + 1, SQ], F32, tag="pa")
                for c in range(NC_K):
                    pscr = ps_scr.tile([P, SQ], F32, tag="pscr")
                    for half in range(SQ // 512):
                        nc.tensor.matmul(
                            pscr[:, ts(half, 512)],
                            kh[ts(hh, DK), ts(c, P)],
                            qh[ts(hh, DK), ts(half, 512)],
                            start=True, stop=True)
                    et = et_p.tile([P, SQ], BF16, tag="et")
                    nc.scalar.activation(et[:], pscr[:], AF.Exp)
                    mt = mt_p.tile([P, SQ], BF16, tag="mt")
                    nc.vector.tensor_mul(mt[:], et[:], maskT[c][:])
                    vsl = (2 * c + hh) * (DK + 1)
                    for half in range(SQ // 512):
                        nc.tensor.matmul(
                            pa[:, ts(half, 512)],
                            vh[:, vsl: vsl + DK + 1],
                            mt[:, ts(half, 512)],
                            start=(c == 0), stop=(c == NC_K - 1))
                    # one proj-group of pair j+1 every few chunks keeps the
                    # PE queue fed while ACT/DVE chew on exp/mask
                    want = (hh * NC_K + c + 1) * nfill // (2 * NC_K)
                    while fi < want:
                        fillers[fi]()
                        fi += 1
                # normalize: rows 0..63 /= row 64 (DVE cannot broadcast
                # along partitions; Pool materializes the recip row)
                rcp = rc_p.tile([1, SQ], F32, tag="rcp")
                nc.vector.reciprocal(rcp[:], pa[DK:DK + 1, :])
                bc = bc_p.tile([DK, SQ], F32, tag="bc")
                nc.gpsimd.partition_broadcast(bc[:], rcp[:])
                nc.vector.tensor_mul(ctT[j][ts(hh, DK), :], pa[0:DK, :], bc[:])

        def emit_oproj():
            # d outer / half inner so consecutive matmuls share the
            # stationary ctT slice (one LDWEIGHTS per d on HW)
            for qt in range(NQT):
                po = ps_scr.tile([P, SQ], F32, tag="pscr")
                for d in range(NJ):
                    for half in range(D // 512):
                        nc.tensor.matmul(
                            po[:, ts(half, 512)],
                            ctT[d][:, ts(qt, P)],
                            wo_sb[d][:, ts(half, 512)],
                            start=(d == 0), stop=False)
                for half in range(D // 512):
                    nc.tensor.matmul(
                        po[:, ts(half, 512)], ones1[:],
                        bo_row[:, ts(half, 512)],
                        start=False, stop=True)
                so = so_p.tile([P, SQ], F32, tag="so")
                nc.scalar.activation(so[:], po[:], AF.Identity)
                nc.sync.dma_start(out_d[ts(qt, P), :], so[:])

        # ---------------- DMA priority order ----------------
        nc.sync.dma_start(ones1[:], ones_d)
        nc.sync.dma_start(bq_sb[:], bq_d.rearrange("(j p) -> p j", p=P))
        emit_wdma(0)
        for d in range(NJ):
            nc.sync.dma_start(qT[d][:], qT_d[ts(d, P), :])
        emit_wdma(1)
        for d in range(NJ):
            nc.sync.dma_start(kT[d][:], kT_d[ts(d, P), :])
        for c in range(3):
            nc.sync.dma_start(maskT[c][:], mT_d[ts(c, P), :])
        for d in range(NJ):
            nc.sync.dma_start(vT[d][:], vT_d[ts(d, P), :])
        for c in range(3, NC_K):
            nc.sync.dma_start(maskT[c][:], mT_d[ts(c, P), :])
        nc.sync.dma_start(bo_row[:], bo_d.rearrange("(o n) -> o n", o=1))
        for d in range(NJ):
            nc.sync.dma_start(wo_sb[d][:], wo_d[ts(d, P), :])

        # ---------------- pipelined schedule ----------------
        for g in proj_groups(0):
            g()
        for j in range(NJ):
            if j + 2 < NJ:
                emit_wdma(j + 2)
            fillers = proj_groups(j + 1) if j + 1 < NJ else ()
            emit_attn(j, fillers)
        emit_oproj()


def make_in_maps(q, k, v, att_mask):
    """Build the 8 per-core input dicts (bf16, pre-transposed)."""
    bf = ml_dtypes.bfloat16
    q = np.asarray(q, dtype=np.float32)
    k = np.asarray(k, dtype=np.float32)
    v = np.asarray(v, dtype=np.float32)
    att_mask = np.asarray(att_mask)
    kT_b = [np.ascontiguousarray(k[b].T).astype(bf) for b in range(B)]
    vT_b = [np.ascontiguousarray(v[b].T).astype(bf) for b in range(B)]
    in_maps = []
    for c in range(N_CORES):
        b, half = divmod(c, 2)
        qs = slice(half * SQ, (half + 1) * SQ)
        in_maps.append({
            "qT": np.ascontiguousarray(q[b, qs, :].T).astype(bf),
            "kT": kT_b[b],
            "vT": vT_b[b],
            "maskT": np.ascontiguousarray(att_mask[b, qs, :].T).astype(bf),
        })
    return in_maps


def make_weights(W_q, b_q, W_k, b_k, W_v, b_v, W_o, b_o):
    bf = ml_dtypes.bfloat16
    W_q = np.asarray(W_q, np.float32)
    W_k = np.asarray(W_k, np.float32)
    W_v = np.asarray(W_v, np.float32)
    W_o = np.asarray(W_o, np.float32)
    b_q = np.asarray(b_q, np.float32)
    b_v = np.asarray(b_v, np.float32)
    b_o = np.asarray(b_o, np.float32)
    scale = 1.0 / np.sqrt(DK)
    # k-bias: adds a per-q constant to every score of a row -> softmax
    # invariant -> dropped.  v-bias: attention weights sum to 1 -> passes
    # through -> fold b_v @ W_o into b_o.
    bo_eff = b_v @ W_o + b_o
    return {
        "wq": (W_q * scale).astype(bf),
        "wk": W_k.astype(bf),
        "wv": W_v.astype(bf),
        "wo": W_o.astype(bf),
        "bq": (b_q * scale).astype(np.float32),
        "bo": bo_eff.astype(bf),
        "ones_row": np.ones((1, P), bf),
    }


_PROG = None


def _get_program():
    global _PROG
    if _PROG is None:
        _PROG = build_program()
    return _PROG


def kernel(q, k, v, att_mask, W_q, b_q, W_k, b_k, W_v, b_v, W_o, b_o,
           **_ignored):
    from concourse.bass_utils import run_bass_kernel_spmd

    nc = _get_program()
    weights = make_weights(W_q, b_q, W_k, b_k, W_v, b_v, W_o, b_o)
    in_maps = [dict(m, **weights) for m in make_in_maps(q, k, v, att_mask)]
    res = run_bass_kernel_spmd(nc, in_maps, core_ids=list(range(N_CORES)))
    out = np.empty((B, S, D), dtype=np.float32)
    for c in range(N_CORES):
        b, half = divmod(c, 2)
        out[b, half * SQ:(half + 1) * SQ, :] = res.results[c]["out"]
    return out


# revision 18
# speedup vs baseline: 1.1961x; 1.1961x over previous
"""Multi-head attention TRN2 Bass kernel, v3 (bf16, fully SBUF-resident).

Problem: B=4, S=2048, D=1024, H=16 heads (DK=64), fp32 reference, random
0/1 attention mask broadcast over heads.

Sharding: 8 cores = (batch b, query-half) pairs.  Core c handles batch
c//2, query rows [(c%2)*1024, (c%2+1)*1024).  K/V projections for the
batch are computed redundantly on the 2 cores sharing a batch; no
collectives, each core writes a disjoint output slice.

v3 design (vs the v1 fp32r DRAM-spilled baseline):
  - All matmul operands bf16 (1 cyc/row like fp32r, half the SBUF/DMA).
    Everything lives in SBUF; no DRAM spills of qh/kh/ct.
  - Head-pair (j) software pipelining: projections for pair j+1 are
    emitted as filler psum-groups interleaved into pair j's attention
    chunk loop, so PE streams projections while ACT/DVE chew on
    exp/mask.  W_v slices span two pairs (256-wide streams) to amortize
    LDWEIGHTS of the vT-stationary v-projection.
  - Algebraic bias folds: k-bias shifts every score of a q-row equally
    -> softmax-invariant -> dropped.  v-bias passes through attention
    (weights sum to 1) -> folded into b_o on host (b_o' = b_v @ W_o +
    b_o).  1/sqrt(dk) and q-bias folded into W_q/b_q on host.
  - Attention (per head): scoresT[k,q] chunks [128,1024] on 2 psum
    banks; exp on ACT with no max-subtraction (scaled scores are O(1));
    mask applied multiplicatively post-exp on DVE in bf16 (2x mode);
    AV keeps d-on-partitions: pa[65, 1024] += [vh|1].T @ mexpT, vh
    stationary (65-column LDWEIGHTS, 512-row streams), denominator in
    row 64.  HW rule respected throughout: at most one open psum
    accumulation group per 2KB bank.
  - Normalize: DVE reciprocal of the denominator row, GPSIMD
    partition_broadcast (DVE cannot broadcast along partitions), DVE
    multiply straight into ctT -- no DMA round trip.
  - Output projection: stationary ctT slice shared by consecutive
    matmuls (d outer), bias via ones-row matmul, ACT drain, DMA out.

PSUM budget (8 banks): scores 2x[128,1024]f32 (4) + AV accum
1x[65,1024]f32 (2) + proj 2x[128,512] (2).
"""

import os
import sys

if "/opt/trn_rl_repo" not in sys.path:
    sys.path.insert(0, "/opt/trn_rl_repo")
os.environ.setdefault("MYCRO_LOCAL_CACHE", "1")

import numpy as np
import ml_dtypes

import concourse.bass as bass
import concourse.bacc as bacc
import concourse.mybir as mybir
import concourse.tile as tile
from concourse.bass import ts

B, S, D, H, DK = 4, 2048, 1024, 16, 64
SQ = S // 2          # q rows per core
P = 128
NC_K = S // P        # 16 k-chunks
NJ = D // P          # 8 feature chunks = head pairs
NQT = SQ // P        # 8 q tiles
N_CORES = 8

F32 = mybir.dt.float32
BF16 = mybir.dt.bfloat16
AF = mybir.ActivationFunctionType


def build_program():
    nc = bacc.Bacc(
        "TRN2",
        target_bir_lowering=False,
        debug=False,
        enable_asserts=False,
    )

    # ---- DRAM I/O (per-core slices; host pre-transposed, bf16) ----
    qT_d = nc.dram_tensor("qT", [D, SQ], BF16, kind="ExternalInput").ap()
    kT_d = nc.dram_tensor("kT", [D, S], BF16, kind="ExternalInput").ap()
    vT_d = nc.dram_tensor("vT", [D, S], BF16, kind="ExternalInput").ap()
    mT_d = nc.dram_tensor("maskT", [S, SQ], BF16, kind="ExternalInput").ap()
    wq_d = nc.dram_tensor("wq", [D, D], BF16, kind="ExternalInput").ap()
    wk_d = nc.dram_tensor("wk", [D, D], BF16, kind="ExternalInput").ap()
    wv_d = nc.dram_tensor("wv", [D, D], BF16, kind="ExternalInput").ap()
    wo_d = nc.dram_tensor("wo", [D, D], BF16, kind="ExternalInput").ap()
    bq_d = nc.dram_tensor("bq", [D], F32, kind="ExternalInput").ap()
    bo_d = nc.dram_tensor("bo", [D], BF16, kind="ExternalInput").ap()
    ones_d = nc.dram_tensor("ones_row", [1, P], BF16, kind="ExternalInput").ap()
    out_d = nc.dram_tensor("out", [SQ, D], F32, kind="ExternalOutput").ap()

    with tile.TileContext(nc) as tc:
        _build(nc, tc, qT_d, kT_d, vT_d, mT_d, wq_d, wk_d, wv_d, wo_d,
               bq_d, bo_d, ones_d, out_d)

    nc.compile()
    return nc


def _build(nc, tc, qT_d, kT_d, vT_d, mT_d, wq_d, wk_d, wv_d, wo_d,
           bq_d, bo_d, ones_d, out_d):
    from contextlib import ExitStack

    with ExitStack() as top:
        # ---------------- persistent SBUF ----------------
        consts = top.enter_context(tc.tile_pool(name="consts", bufs=1))
        ones1 = consts.tile([1, P], BF16, tag="ones1")
        bo_row = consts.tile([1, D], BF16, tag="bo_row")
        bq_sb = consts.tile([P, NJ], F32, tag="bq_sb")

        inp = top.enter_context(tc.tile_pool(name="inp", bufs=1))
        qT = [inp.tile([P, SQ], BF16, tag=f"qT{d}", name=f"qT{d}")
              for d in range(NJ)]
        kT = [inp.tile([P, S], BF16, tag=f"kT{d}", name=f"kT{d}")
              for d in range(NJ)]
        vT = [inp.tile([P, S], BF16, tag=f"vT{d}", name=f"vT{d}")
              for d in range(NJ)]
        maskT = [inp.tile([P, SQ], BF16, tag=f"mT{c}", name=f"mT{c}")
                 for c in range(NC_K)]
        wo_sb = [inp.tile([P, D], BF16, tag=f"wo{d}", name=f"wo{d}")
                 for d in range(NJ)]
        ctT = [inp.tile([P, SQ], BF16, tag=f"ctT{d}", name=f"ctT{d}")
               for d in range(NJ)]

        # ---------------- ring pools (head-pair pipelined) ----------------
        wsl = top.enter_context(tc.tile_pool(name="wsl", bufs=2))
        qh_p = top.enter_context(tc.tile_pool(name="qh", bufs=2))
        kh_p = top.enter_context(tc.tile_pool(name="kh", bufs=2))
        vh_p = top.enter_context(tc.tile_pool(name="vh", bufs=2))
        et_p = top.enter_context(tc.tile_pool(name="et", bufs=2))
        mt_p = top.enter_context(tc.tile_pool(name="mt", bufs=3))
        rc_p = top.enter_context(tc.tile_pool(name="rc", bufs=1))
        bc_p = top.enter_context(tc.tile_pool(name="bc", bufs=1))
        so_p = top.enter_context(tc.tile_pool(name="so", bufs=1))

        ps_scr = top.enter_context(
            tc.tile_pool(name="ps_scr", bufs=2, space="PSUM"))
        ps_av = top.enter_context(
            tc.tile_pool(name="ps_av", bufs=1, space="PSUM"))
        ps_pr = top.enter_context(
            tc.tile_pool(name="ps_pr", bufs=2, space="PSUM"))

        # W slice view: DRAM W[1024, 1024] rows (dc p), cols j*128+f
        # -> SBUF [p, dc, f] = [128, 8, width]
        def w_slice_ap(w_d, j, width=P):
            return w_d.rearrange("(dc p) f -> p dc f", p=P)[
                :, :, j * P: j * P + width]

        state = {}

        def emit_wdma(j):
            """Prefetch W_q/W_k slices for pair j; W_v slice covers pairs
            (j, j+1) and is fetched on even j only (v-proj streams 256-wide
            to amortize LDWEIGHTS of the vT-stationary)."""
            wq_t = wsl.tile([P, D], BF16, tag="wq_sl")
            wk_t = wsl.tile([P, D], BF16, tag="wk_sl")
            nc.sync.dma_start(
                wq_t[:].rearrange("p (dc f) -> p dc f", f=P),
                w_slice_ap(wq_d, j))
            nc.sync.dma_start(
                wk_t[:].rearrange("p (dc f) -> p dc f", f=P),
                w_slice_ap(wk_d, j))
            if j % 2 == 0:
                wv_t = wsl.tile([P, 2 * D], BF16, tag="wv_sl")
                nc.sync.dma_start(
                    wv_t[:].rearrange("p (dc f) -> p dc f", f=2 * P),
                    w_slice_ap(wv_d, j, width=2 * P))
            else:
                wv_t = None
            state[("w", j)] = (wq_t, wk_t, wv_t)

        def proj_groups(j):
            """Projections for pair j as a list of psum-group closures, so
            they can be interleaved into the previous pair's attention.
            Even j also produces pair j+1's v-projection (256-wide)."""
            wq_t, wk_t, wv_t = state.pop(("w", j))
            qh = qh_p.tile([P, SQ], BF16, tag="qh")
            kh = kh_p.tile([P, S], BF16, tag="kh")
            if j % 2 == 0:
                vh = [vh_p.tile([P, 2 * (DK + 1)], BF16, tag=f"vh{c}",
                                name=f"vh{c}_{j}") for c in range(NC_K)]
                vh1 = [vh_p.tile([P, 2 * (DK + 1)], BF16, tag=f"vh{c}",
                                 name=f"vh{c}_{j + 1}") for c in range(NC_K)]
                state[("vh", j + 1)] = vh1
            else:
                vh = state.pop(("vh", j))
                vh1 = None
            state[j] = (qh, kh, vh)
            groups = []

            # q-proj: out[feat 128, q 512] x2, contraction over 8 d-chunks
            def qgroup(half):
                pq = ps_pr.tile([P, 512], F32, tag="ppr")
                for d in range(NJ):
                    nc.tensor.matmul(
                        pq[:], wq_t[:, ts(d, P)], qT[d][:, ts(half, 512)],
                        start=(d == 0), stop=(d == NJ - 1))
                nc.vector.tensor_scalar_add(
                    qh[:, ts(half, 512)], pq[:], bq_sb[:, ts(j, 1)])

            # k-proj: out[feat 128, k 512] x4 (no bias: softmax-invariant)
            def kgroup(half):
                pk = ps_pr.tile([P, 512], F32, tag="ppr")
                for d in range(NJ):
                    nc.tensor.matmul(
                        pk[:], wk_t[:, ts(d, P)], kT[d][:, ts(half, 512)],
                        start=(d == 0), stop=(d == NJ - 1))
                nc.vector.tensor_copy(kh[:, ts(half, 512)], pk[:])

            # v-proj (even j): out[kpos 128, 256 feat spanning pairs
            # j, j+1] per k-chunk; 2 sequential psum groups per tile (one
            # open accumulation group per bank).  v-bias folded into b_o.
            def vgroup(g):
                pv = ps_pr.tile([P, 512], F32, tag="ppr")
                for cc in range(2):
                    c = g * 2 + cc
                    for d in range(NJ):
                        nc.tensor.matmul(
                            pv[:, cc * 256:(cc + 1) * 256],
                            vT[d][:, ts(c, P)],
                            wv_t[:, d * 256:(d + 1) * 256],
                            start=(d == 0), stop=(d == NJ - 1))
                    for pj, vdst in ((0, vh), (1, vh1)):
                        dst = vdst[c].rearrange("p (h w) -> p h w", w=DK + 1)
                        seg = pv[:, cc * 256 + pj * P: cc * 256 + (pj + 1) * P]
                        nc.vector.tensor_copy(
                            dst[:, :, 0:DK],
                            seg.rearrange("p (h w) -> p h w", w=DK))
                        nc.gpsimd.memset(dst[:, :, DK:DK + 1], 1.0)

            import functools
            for half in range(S // 512):
                groups.append(functools.partial(kgroup, half))
            for half in range(SQ // 512):
                groups.append(functools.partial(qgroup, half))
            if j % 2 == 0:
                for g in range(NC_K // 2):
                    groups.append(functools.partial(vgroup, g))
            return groups

        def emit_attn(j, fillers=()):
            """Attention for pair j; fillers = next pair's proj groups,
            interleaved into the chunk stream to keep PE fed while ACT/DVE
            work on exp/mask."""
            fillers = list(fillers)
            qh, kh, vh = state.pop(j)
            nfill = len(fillers)
            fi = 0
            for hh in range(2):
                # pa rows 0..63 = unnormalized out^T, row 64 = softmax
                # denominators (vh column 64 is all-ones); the two 512-wide
                # halves are separate accumulation groups, one per bank.
                pa = ps_av.tile([DK + 1, SQ], F32, tag="pa")
                for c in range(NC_K):
                    pscr = ps_scr.tile([P, SQ], F32, tag="pscr")
                    for half in range(SQ // 512):
                        nc.tensor.matmul(
                            pscr[:, ts(half, 512)],
                            kh[ts(hh, DK), ts(c, P)],
                            qh[ts(hh, DK), ts(half, 512)],
                            start=True, stop=True)
                    et = et_p.tile([P, SQ], BF16, tag="et")
                    nc.scalar.activation(et[:], pscr[:], AF.Exp)
                    mt = mt_p.tile([P, SQ], BF16, tag="mt")
                    nc.vector.tensor_mul(mt[:], et[:], maskT[c][:])
                    for half in range(SQ // 512):
                        nc.tensor.matmul(
                            pa[:, ts(half, 512)],
                            vh[c][:, hh * (DK + 1): (hh + 1) * (DK + 1)],
                            mt[:, ts(half, 512)],
                            start=(c == 0), stop=(c == NC_K - 1))
                    # one proj-group of pair j+1 every few chunks keeps the
                    # PE queue fed while ACT/DVE chew on exp/mask
                    want = (hh * NC_K + c + 1) * nfill // (2 * NC_K)
                    while fi < want:
                        fillers[fi]()
                        fi += 1
                # normalize: rows 0..63 /= row 64 (DVE cannot broadcast
                # along partitions; Pool materializes the recip row)
                rcp = rc_p.tile([1, SQ], F32, tag="rcp")
                nc.vector.reciprocal(rcp[:], pa[DK:DK + 1, :])
                bc = bc_p.tile([DK, SQ], F32, tag="bc")
                nc.gpsimd.partition_broadcast(bc[:], rcp[:])
                nc.vector.tensor_mul(ctT[j][ts(hh, DK), :], pa[0:DK, :], bc[:])

        def emit_oproj():
            # d outer / half inner so consecutive matmuls share the
            # stationary ctT slice (one LDWEIGHTS per d on HW)
            for qt in range(NQT):
                po = ps_scr.tile([P, SQ], F32, tag="pscr")
                for d in range(NJ):
                    for half in range(D // 512):
                        nc.tensor.matmul(
                            po[:, ts(half, 512)],
                            ctT[d][:, ts(qt, P)],
                            wo_sb[d][:, ts(half, 512)],
                            start=(d == 0), stop=False)
                for half in range(D // 512):
                    nc.tensor.matmul(
                        po[:, ts(half, 512)], ones1[:],
                        bo_row[:, ts(half, 512)],
                        start=False, stop=True)
                so = so_p.tile([P, SQ], F32, tag="so")
                nc.scalar.activation(so[:], po[:], AF.Identity)
                nc.sync.dma_start(out_d[ts(qt, P), :], so[:])

        # ---------------- DMA priority order ----------------
        nc.sync.dma_start(ones1[:], ones_d)
        nc.sync.dma_start(bq_sb[:], bq_d.rearrange("(j p) -> p j", p=P))
        emit_wdma(0)
        for d in range(NJ):
            nc.sync.dma_start(qT[d][:], qT_d[ts(d, P), :])
        emit_wdma(1)
        for d in range(NJ):
            nc.sync.dma_start(kT[d][:], kT_d[ts(d, P), :])
        for c in range(3):
            nc.sync.dma_start(maskT[c][:], mT_d[ts(c, P), :])
        for d in range(NJ):
            nc.sync.dma_start(vT[d][:], vT_d[ts(d, P), :])
        for c in range(3, NC_K):
            nc.sync.dma_start(maskT[c][:], mT_d[ts(c, P), :])
        nc.sync.dma_start(bo_row[:], bo_d.rearrange("(o n) -> o n", o=1))
        for d in range(NJ):
            nc.sync.dma_start(wo_sb[d][:], wo_d[ts(d, P), :])

        # ---------------- pipelined schedule ----------------
        for g in proj_groups(0):
            g()
        for j in range(NJ):
            if j + 2 < NJ:
                emit_wdma(j + 2)
            fillers = proj_groups(j + 1) if j + 1 < NJ else ()
            emit_attn(j, fillers)
        emit_oproj()


def make_in_maps(q, k, v, att_mask):
    """Build the 8 per-core input dicts (bf16, pre-transposed)."""
    bf = ml_dtypes.bfloat16
    q = np.asarray(q, dtype=np.float32)
    k = np.asarray(k, dtype=np.float32)
    v = np.asarray(v, dtype=np.float32)
    att_mask = np.asarray(att_mask)
    kT_b = [np.ascontiguousarray(k[b].T).astype(bf) for b in range(B)]
    vT_b = [np.ascontiguousarray(v[b].T).astype(bf) for b in range(B)]
    in_maps = []
    for c in range(N_CORES):
        b, half = divmod(c, 2)
        qs = slice(half * SQ, (half + 1) * SQ)
        in_maps.append({
            "qT": np.ascontiguousarray(q[b, qs, :].T).astype(bf),
            "kT": kT_b[b],
            "vT": vT_b[b],
            "maskT": np.ascontiguousarray(att_mask[b, qs, :].T).astype(bf),
        })
    return in_maps


def make_weights(W_q, b_q, W_k, b_k, W_v, b_v, W_o, b_o):
    bf = ml_dtypes.bfloat16
    W_q = np.asarray(W_q, np.float32)
    W_k = np.asarray(W_k, np.float32)
    W_v = np.asarray(W_v, np.float32)
    W_o = np.asarray(W_o, np.float32)
    b_q = np.asarray(b_q, np.float32)
    b_v = np.asarray(b_v, np.float32)
    b_o = np.asarray(b_o, np.float32)
    scale = 1.0 / np.sqrt(DK)
    # k-bias: adds a per-q constant to every score of a row -> softmax
    # invariant -> dropped.  v-bias: attention weights sum to 1 -> passes
    # through -> fold b_v @ W_o into b_o.
    bo_eff = b_v @ W_o + b_o
    return {
        "wq": (W_q * scale).astype(bf),
        "wk": W_k.astype(bf),
        "wv": W_v.astype(bf),
        "wo": W_o.astype(bf),
        "bq": (b_q * scale).astype(np.float32),
        "bo": bo_eff.astype(bf),
        "ones_row": np.ones((1, P), bf),
    }


_PROG = None


def _get_program():
    global _PROG
    if _PROG is None:
        _PROG = build_program()
    return _PROG


def kernel(q, k, v, att_mask, W_q, b_q, W_k, b_k, W_v, b_v, W_o, b_o,
           **_ignored):
    from concourse.bass_utils import run_bass_kernel_spmd

    nc = _get_program()
    weights = make_weights(W_q, b_q, W_k, b_k, W_v, b_v, W_o, b_o)
    in_maps = [dict(m, **weights) for m in make_in_maps(q, k, v, att_mask)]
    res = run_bass_kernel_spmd(nc, in_maps, core_ids=list(range(N_CORES)))
    out = np.empty((B, S, D), dtype=np.float32)
    for c in range(N_CORES):
        b, half = divmod(c, 2)
        out[b, half * SQ:(half + 1) * SQ, :] = res.results[c]["out"]
    return out


# revision 19
# speedup vs baseline: 1.5185x; 1.2695x over previous
"""Multi-head attention TRN2 Bass kernel, v3 (bf16, fully SBUF-resident).

Problem: B=4, S=2048, D=1024, H=16 heads (DK=64), fp32 reference, random
0/1 attention mask broadcast over heads.

Sharding: 8 cores = (batch b, query-half) pairs.  Core c handles batch
c//2, query rows [(c%2)*1024, (c%2+1)*1024).  K/V projections for the
batch are computed redundantly on the 2 cores sharing a batch; no
collectives, each core writes a disjoint output slice.

v3 design (vs the v1 fp32r DRAM-spilled baseline):
  - All matmul operands bf16 (1 cyc/row like fp32r, half the SBUF/DMA).
    Everything lives in SBUF; no DRAM spills of qh/kh/ct.
  - Head-pair (j) software pipelining: projections for pair j+1 are
    emitted as filler psum-groups interleaved into pair j's attention
    chunk loop, so PE streams projections while ACT/DVE chew on
    exp/mask.  W_v slices span two pairs (256-wide streams) to amortize
    LDWEIGHTS of the vT-stationary v-projection.
  - Algebraic bias folds: k-bias shifts every score of a q-row equally
    -> softmax-invariant -> dropped.  v-bias passes through attention
    (weights sum to 1) -> folded into b_o on host (b_o' = b_v @ W_o +
    b_o).  1/sqrt(dk) and q-bias folded into W_q/b_q on host.
  - Attention (per head): scoresT[k,q] chunks [128,1024] on 2 psum
    banks; exp on ACT with no max-subtraction (scaled scores are O(1));
    mask applied multiplicatively post-exp on DVE in bf16 (2x mode);
    AV keeps d-on-partitions: pa[65, 1024] += [vh|1].T @ mexpT, vh
    stationary (65-column LDWEIGHTS, 512-row streams), denominator in
    row 64.  HW rule respected throughout: at most one open psum
    accumulation group per 2KB bank.
  - Normalize: DVE reciprocal of the denominator row, GPSIMD
    partition_broadcast (DVE cannot broadcast along partitions), DVE
    multiply straight into ctT -- no DMA round trip.
  - Output projection: stationary ctT slice shared by consecutive
    matmuls (d outer), bias via ones-row matmul, ACT drain, DMA out.

PSUM budget (8 banks): scores 2x[128,1024]f32 (4) + AV accum
1x[65,1024]f32 (2) + proj 2x[128,512] (2).
"""

import os
import sys

if "/opt/trn_rl_repo" not in sys.path:
    sys.path.insert(0, "/opt/trn_rl_repo")
os.environ.setdefault("MYCRO_LOCAL_CACHE", "1")

import numpy as np
import ml_dtypes

import concourse.bass as bass
import concourse.bacc as bacc
import concourse.mybir as mybir
import concourse.tile as tile
from concourse.bass import ts

B, S, D, H, DK = 4, 2048, 1024, 16, 64
SQ = S // 2          # q rows per core
P = 128
NC_K = S // P        # 16 k-chunks
NJ = D // P          # 8 feature chunks = head pairs
NQT = SQ // P        # 8 q tiles
N_CORES = 8

F32 = mybir.dt.float32
BF16 = mybir.dt.bfloat16
AF = mybir.ActivationFunctionType


def build_program():
    nc = bacc.Bacc(
        "TRN2",
        target_bir_lowering=False,
        debug=False,
        enable_asserts=False,
    )

    # ---- DRAM I/O (per-core slices; host pre-transposed, bf16) ----
    qT_d = nc.dram_tensor("qT", [D, SQ], BF16, kind="ExternalInput").ap()
    kT_d = nc.dram_tensor("kT", [D, S], BF16, kind="ExternalInput").ap()
    vT_d = nc.dram_tensor("vT", [D, S], BF16, kind="ExternalInput").ap()
    mT_d = nc.dram_tensor("maskT", [S, SQ], BF16, kind="ExternalInput").ap()
    wq_d = nc.dram_tensor("wq", [D, D], BF16, kind="ExternalInput").ap()
    wk_d = nc.dram_tensor("wk", [D, D], BF16, kind="ExternalInput").ap()
    wv_d = nc.dram_tensor("wv", [D, D], BF16, kind="ExternalInput").ap()
    wo_d = nc.dram_tensor("wo", [D, D], BF16, kind="ExternalInput").ap()
    bq_d = nc.dram_tensor("bq", [D], F32, kind="ExternalInput").ap()
    bo_d = nc.dram_tensor("bo", [D], BF16, kind="ExternalInput").ap()
    ones_d = nc.dram_tensor("ones_row", [1, P], BF16, kind="ExternalInput").ap()
    out_d = nc.dram_tensor("out", [SQ, D], F32, kind="ExternalOutput").ap()

    with tile.TileContext(nc) as tc:
        _build(nc, tc, qT_d, kT_d, vT_d, mT_d, wq_d, wk_d, wv_d, wo_d,
               bq_d, bo_d, ones_d, out_d)

    nc.compile()
    return nc


def _build(nc, tc, qT_d, kT_d, vT_d, mT_d, wq_d, wk_d, wv_d, wo_d,
           bq_d, bo_d, ones_d, out_d):
    from contextlib import ExitStack

    with ExitStack() as top:
        # ---------------- persistent SBUF ----------------
        consts = top.enter_context(tc.tile_pool(name="consts", bufs=1))
        ones1 = consts.tile([1, P], BF16, tag="ones1")
        bo_row = consts.tile([1, D], BF16, tag="bo_row")
        bq_sb = consts.tile([P, NJ], F32, tag="bq_sb")

        inp = top.enter_context(tc.tile_pool(name="inp", bufs=1))
        qT = [inp.tile([P, SQ], BF16, tag=f"qT{d}", name=f"qT{d}")
              for d in range(NJ)]
        kT = [inp.tile([P, S], BF16, tag=f"kT{d}", name=f"kT{d}")
              for d in range(NJ)]
        vT = [inp.tile([P, S], BF16, tag=f"vT{d}", name=f"vT{d}")
              for d in range(NJ)]
        maskT = [inp.tile([P, SQ], BF16, tag=f"mT{c}", name=f"mT{c}")
                 for c in range(NC_K)]
        wo_sb = [inp.tile([P, D], BF16, tag=f"wo{d}", name=f"wo{d}")
                 for d in range(NJ)]
        ctT = [inp.tile([P, SQ], BF16, tag=f"ctT{d}", name=f"ctT{d}")
               for d in range(NJ)]

        # ---------------- ring pools (head-pair pipelined) ----------------
        wsl = top.enter_context(tc.tile_pool(name="wsl", bufs=2))
        qh_p = top.enter_context(tc.tile_pool(name="qh", bufs=2))
        kh_p = top.enter_context(tc.tile_pool(name="kh", bufs=2))
        vh_p = top.enter_context(tc.tile_pool(name="vh", bufs=2))
        et_p = top.enter_context(tc.tile_pool(name="et", bufs=3))
        mt_p = top.enter_context(tc.tile_pool(name="mt", bufs=3))
        rc_p = top.enter_context(tc.tile_pool(name="rc", bufs=1))
        bc_p = top.enter_context(tc.tile_pool(name="bc", bufs=1))
        so_p = top.enter_context(tc.tile_pool(name="so", bufs=1))

        ps_scr = top.enter_context(
            tc.tile_pool(name="ps_scr", bufs=2, space="PSUM"))
        ps_av = top.enter_context(
            tc.tile_pool(name="ps_av", bufs=1, space="PSUM"))
        ps_pr = top.enter_context(
            tc.tile_pool(name="ps_pr", bufs=2, space="PSUM"))

        # W slice view: DRAM W[1024, 1024] rows (dc p), cols j*128+f
        # -> SBUF [p, dc, f] = [128, 8, width]
        def w_slice_ap(w_d, j, width=P):
            return w_d.rearrange("(dc p) f -> p dc f", p=P)[
                :, :, j * P: j * P + width]

        state = {}

        def emit_wdma(j):
            """Prefetch W_q/W_k slices for pair j; W_v slice covers pairs
            (j, j+1) and is fetched on even j only (v-proj streams 256-wide
            to amortize LDWEIGHTS of the vT-stationary)."""
            wq_t = wsl.tile([P, D], BF16, tag="wq_sl")
            wk_t = wsl.tile([P, D], BF16, tag="wk_sl")
            nc.sync.dma_start(
                wq_t[:].rearrange("p (dc f) -> p dc f", f=P),
                w_slice_ap(wq_d, j))
            nc.sync.dma_start(
                wk_t[:].rearrange("p (dc f) -> p dc f", f=P),
                w_slice_ap(wk_d, j))
            if j % 2 == 0:
                wv_t = wsl.tile([P, 2 * D], BF16, tag="wv_sl")
                nc.sync.dma_start(
                    wv_t[:].rearrange("p (dc f) -> p dc f", f=2 * P),
                    w_slice_ap(wv_d, j, width=2 * P))
            else:
                wv_t = None
            state[("w", j)] = (wq_t, wk_t, wv_t)

        def proj_groups(j):
            """Projections for pair j as a list of psum-group closures, so
            they can be interleaved into the previous pair's attention.
            Even j also produces pair j+1's v-projection (256-wide)."""
            wq_t, wk_t, wv_t = state.pop(("w", j))
            qh = qh_p.tile([P, SQ], BF16, tag="qh")
            kh = kh_p.tile([P, S], BF16, tag="kh")
            if j % 2 == 0:
                vh = [vh_p.tile([P, 2 * (DK + 1)], BF16, tag=f"vh{c}",
                                name=f"vh{c}_{j}") for c in range(NC_K)]
                vh1 = [vh_p.tile([P, 2 * (DK + 1)], BF16, tag=f"vh{c}",
                                 name=f"vh{c}_{j + 1}") for c in range(NC_K)]
                state[("vh", j + 1)] = vh1
            else:
                vh = state.pop(("vh", j))
                vh1 = None
            state[j] = (qh, kh, vh)
            groups = []

            # q-proj: out[feat 128, q 512] x2, contraction over 8 d-chunks
            def qgroup(half):
                pq = ps_pr.tile([P, 512], F32, tag="ppr")
                for d in range(NJ):
                    nc.tensor.matmul(
                        pq[:], wq_t[:, ts(d, P)], qT[d][:, ts(half, 512)],
                        start=(d == 0), stop=(d == NJ - 1))
                nc.vector.tensor_scalar_add(
                    qh[:, ts(half, 512)], pq[:], bq_sb[:, ts(j, 1)])

            # k-proj: out[feat 128, k 512] x4 (no bias: softmax-invariant)
            def kgroup(half):
                pk = ps_pr.tile([P, 512], F32, tag="ppr")
                for d in range(NJ):
                    nc.tensor.matmul(
                        pk[:], wk_t[:, ts(d, P)], kT[d][:, ts(half, 512)],
                        start=(d == 0), stop=(d == NJ - 1))
                nc.vector.tensor_copy(kh[:, ts(half, 512)], pk[:])

            # v-proj (even j): out[kpos 128, 256 feat spanning pairs
            # j, j+1] per k-chunk; 2 sequential psum groups per tile (one
            # open accumulation group per bank).  v-bias folded into b_o.
            def vgroup(g):
                pv = ps_pr.tile([P, 512], F32, tag="ppr")
                for cc in range(2):
                    c = g * 2 + cc
                    for d in range(NJ):
                        nc.tensor.matmul(
                            pv[:, cc * 256:(cc + 1) * 256],
                            vT[d][:, ts(c, P)],
                            wv_t[:, d * 256:(d + 1) * 256],
                            start=(d == 0), stop=(d == NJ - 1))
                    for pj, vdst in ((0, vh), (1, vh1)):
                        dst = vdst[c].rearrange("p (h w) -> p h w", w=DK + 1)
                        seg = pv[:, cc * 256 + pj * P: cc * 256 + (pj + 1) * P]
                        nc.vector.tensor_copy(
                            dst[:, :, 0:DK],
                            seg.rearrange("p (h w) -> p h w", w=DK))
                        nc.gpsimd.memset(dst[:, :, DK:DK + 1], 1.0)

            import functools
            for half in range(S // 512):
                groups.append(functools.partial(kgroup, half))
            for half in range(SQ // 512):
                groups.append(functools.partial(qgroup, half))
            if j % 2 == 0:
                for g in range(NC_K // 2):
                    groups.append(functools.partial(vgroup, g))
            return groups

        def emit_attn(j, fillers=()):
            """Attention for pair j; fillers = next pair's proj groups,
            interleaved into the chunk stream to keep PE fed while ACT/DVE
            work on exp/mask."""
            fillers = list(fillers)
            qh, kh, vh = state.pop(j)
            nfill = len(fillers)
            fi = 0
            for hh in range(2):
                # pa rows 0..63 = unnormalized out^T, row 64 = softmax
                # denominators (vh column 64 is all-ones); the two 512-wide
                # halves are separate accumulation groups, one per bank.
                pa = ps_av.tile([DK + 1, SQ], F32, tag="pa")
                for c in range(NC_K):
                    pscr = ps_scr.tile([P, SQ], F32, tag="pscr")
                    for half in range(SQ // 512):
                        nc.tensor.matmul(
                            pscr[:, ts(half, 512)],
                            kh[ts(hh, DK), ts(c, P)],
                            qh[ts(hh, DK), ts(half, 512)],
                            start=True, stop=True)
                    et = et_p.tile([P, SQ], BF16, tag="et")
                    nc.scalar.activation(et[:], pscr[:], AF.Exp)
                    mt = mt_p.tile([P, SQ], BF16, tag="mt")
                    nc.vector.tensor_mul(mt[:], et[:], maskT[c][:])
                    for half in range(SQ // 512):
                        nc.tensor.matmul(
                            pa[:, ts(half, 512)],
                            vh[c][:, hh * (DK + 1): (hh + 1) * (DK + 1)],
                            mt[:, ts(half, 512)],
                            start=(c == 0), stop=(c == NC_K - 1))
                    # one proj-group of pair j+1 every few chunks keeps the
                    # PE queue fed while ACT/DVE chew on exp/mask
                    want = (hh * NC_K + c + 1) * nfill // (2 * NC_K)
                    while fi < want:
                        fillers[fi]()
                        fi += 1
                # normalize: rows 0..63 /= row 64 (DVE cannot broadcast
                # along partitions; Pool materializes the recip row)
                rcp = rc_p.tile([1, SQ], F32, tag="rcp")
                nc.vector.reciprocal(rcp[:], pa[DK:DK + 1, :])
                bc = bc_p.tile([DK, SQ], F32, tag="bc")
                nc.gpsimd.partition_broadcast(bc[:], rcp[:])
                nc.vector.tensor_mul(ctT[j][ts(hh, DK), :], pa[0:DK, :], bc[:])

        def emit_oproj():
            # d outer / half inner so consecutive matmuls share the
            # stationary ctT slice (one LDWEIGHTS per d on HW)
            for qt in range(NQT):
                po = ps_scr.tile([P, SQ], F32, tag="pscr")
                for d in range(NJ):
                    for half in range(D // 512):
                        nc.tensor.matmul(
                            po[:, ts(half, 512)],
                            ctT[d][:, ts(qt, P)],
                            wo_sb[d][:, ts(half, 512)],
                            start=(d == 0), stop=False)
                for half in range(D // 512):
                    nc.tensor.matmul(
                        po[:, ts(half, 512)], ones1[:],
                        bo_row[:, ts(half, 512)],
                        start=False, stop=True)
                so = so_p.tile([P, SQ], F32, tag="so")
                nc.scalar.activation(so[:], po[:], AF.Identity)
                nc.sync.dma_start(out_d[ts(qt, P), :], so[:])

        # ---------------- DMA priority order ----------------
        nc.sync.dma_start(ones1[:], ones_d)
        nc.sync.dma_start(bq_sb[:], bq_d.rearrange("(j p) -> p j", p=P))
        # wq slice 0 + qT first: they alone gate the first projection
        wq_t0 = wsl.tile([P, D], BF16, tag="wq_sl", name="wq_t0")
        nc.sync.dma_start(
            wq_t0[:].rearrange("p (dc f) -> p dc f", f=P),
            w_slice_ap(wq_d, 0))
        for d in range(NJ):
            nc.sync.dma_start(qT[d][:], qT_d[ts(d, P), :])
        wk_t0 = wsl.tile([P, D], BF16, tag="wk_sl", name="wk_t0")
        nc.sync.dma_start(
            wk_t0[:].rearrange("p (dc f) -> p dc f", f=P),
            w_slice_ap(wk_d, 0))
        wv_t0 = wsl.tile([P, 2 * D], BF16, tag="wv_sl", name="wv_t0")
        nc.sync.dma_start(
            wv_t0[:].rearrange("p (dc f) -> p dc f", f=2 * P),
            w_slice_ap(wv_d, 0, width=2 * P))
        state[("w", 0)] = (wq_t0, wk_t0, wv_t0)
        emit_wdma(1)
        for d in range(NJ):
            nc.sync.dma_start(kT[d][:], kT_d[ts(d, P), :])
        for c in range(3):
            nc.sync.dma_start(maskT[c][:], mT_d[ts(c, P), :])
        for d in range(NJ):
            nc.sync.dma_start(vT[d][:], vT_d[ts(d, P), :])
        for c in range(3, NC_K):
            nc.sync.dma_start(maskT[c][:], mT_d[ts(c, P), :])
        nc.sync.dma_start(bo_row[:], bo_d.rearrange("(o n) -> o n", o=1))
        for d in range(NJ):
            nc.sync.dma_start(wo_sb[d][:], wo_d[ts(d, P), :])

        # ---------------- pipelined schedule ----------------
        for g in proj_groups(0):
            g()
        for j in range(NJ):
            if j + 2 < NJ:
                emit_wdma(j + 2)
            fillers = proj_groups(j + 1) if j + 1 < NJ else ()
            emit_attn(j, fillers)
        emit_oproj()


def make_in_maps(q, k, v, att_mask):
    """Build the 8 per-core input dicts (bf16, pre-transposed)."""
    bf = ml_dtypes.bfloat16
    q = np.asarray(q, dtype=np.float32)
    k = np.asarray(k, dtype=np.float32)
    v = np.asarray(v, dtype=np.float32)
    att_mask = np.asarray(att_mask)
    kT_b = [np.ascontiguousarray(k[b].T).astype(bf) for b in range(B)]
    vT_b = [np.ascontiguousarray(v[b].T).astype(bf) for b in range(B)]
    in_maps = []
    for c in range(N_CORES):
        b, half = divmod(c, 2)
        qs = slice(half * SQ, (half + 1) * SQ)
        in_maps.append({
            "qT": np.ascontiguousarray(q[b, qs, :].T).astype(bf),
            "kT": kT_b[b],
            "vT": vT_b[b],
            "maskT": np.ascontiguousarray(att_mask[b, qs, :].T).astype(bf),
        })
    return in_maps


def make_weights(W_q, b_q, W_k, b_k, W_v, b_v, W_o, b_o):
    bf = ml_dtypes.bfloat16
    W_q = np.asarray(W_q, np.float32)
    W_k = np.asarray(W_k, np.float32)
    W_v = np.asarray(W_v, np.float32)
    W_o = np.asarray(W_o, np.float32)
    b_q = np.asarray(b_q, np.float32)
    b_v = np.asarray(b_v, np.float32)
    b_o = np.asarray(b_o, np.float32)
    scale = 1.0 / np.sqrt(DK)
    # k-bias: adds a per-q constant to every score of a row -> softmax
    # invariant -> dropped.  v-bias: attention weights sum to 1 -> passes
    # through -> fold b_v @ W_o into b_o.
    bo_eff = b_v @ W_o + b_o
    return {
        "wq": (W_q * scale).astype(bf),
        "wk": W_k.astype(bf),
        "wv": W_v.astype(bf),
        "wo": W_o.astype(bf),
        "bq": (b_q * scale).astype(np.float32),
        "bo": bo_eff.astype(bf),
        "ones_row": np.ones((1, P), bf),
    }


_PROG = None


def _get_program():
    global _PROG
    if _PROG is None:
        _PROG = build_program()
    return _PROG


def kernel(q, k, v, att_mask, W_q, b_q, W_k, b_k, W_v, b_v, W_o, b_o,
           **_ignored):
    from concourse.bass_utils import run_bass_kernel_spmd

    nc = _get_program()
    weights = make_weights(W_q, b_q, W_k, b_k, W_v, b_v, W_o, b_o)
    in_maps = [dict(m, **weights) for m in make_in_maps(q, k, v, att_mask)]
    res = run_bass_kernel_spmd(nc, in_maps, core_ids=list(range(N_CORES)))
    out = np.empty((B, S, D), dtype=np.float32)
    for c in range(N_CORES):
        b, half = divmod(c, 2)
        out[b, half * SQ:(half + 1) * SQ, :] = res.results[c]["out"]
    return out
